# revision 1
# baseline (speedup 1.0000x reference)
"""Trainium2 Bass kernel for windowed (local) causal self-attention.

Reference computation (per batch element, fp32):
    q = x @ Wq.T + bq ; k = x @ Wk.T + bk ; v = x @ Wv.T + bv
    per non-overlapping window of 256 tokens:
        attn = softmax(causal_mask(q k^T * HEAD_DIM**-0.5))
        out  = attn @ v
    o = out @ Wo.T + bo + x

Sharding: data-parallel over (batch, window): 64 window-blocks of 256
tokens -> 8 cores x 8 windows.  Weights replicated.

Per-core kernel strategy:
  - all four transposed weights (W.T, [e_in, e_out]) resident in SBUF as
    float32r (PE matmul dtype: 1 cycle/row at N>=256, measured
    bit-identical to the PE's fp32 matmul, which runs 4 cycles/row).
  - x is transposed on the host and streamed per window as xT [E, 256]
    (kills 128 PE transposes + DVE evacs per core).
  - per window: qT/kT = Wm.T^T @ xT in [e_out, t] layout (bias fused into
    the ACT psum evacuation); scores accumulate over 8 K-tiles; causal
    mask added from an inline constant during psum evac; scale+exp+row-sum
    fused in one ACT op (accum_out); attn normalized by 1/sum on DVE,
    PE-transposed to attnT; v token-major; outT = v^T @ attnT (+bv in ACT
    evac); o = outT^T @ Wo.T (+bo via K=1 ones-matmul into the same psum
    group) + x residual fused into the DVE evacuation.
  - window-0 weight DMA (16MB) is the critical serial phase: loads are
    chunked and interleaved with compute emission, with tiny PE warmup
    transposes paced by arriving chunks to keep the HAM clock at 2.4GHz;
    output stores are deferred behind the next window's loads to avoid
    head-of-line blocking on the sync DMA queue.
"""
import sys

sys.path.insert(0, "/opt/trn_rl_repo")

import numpy as np

import concourse.bass as bass
import concourse.bacc as bacc
import concourse.mybir as mybir
import concourse.tile as tile
from concourse.bass_utils import run_bass_kernel_spmd

F32 = mybir.dt.float32
F32R = mybir.dt.float32r
AF = mybir.ActivationFunctionType

E = 1024          # embed dim
ET = E // 128     # e-tiles
W = 256           # window size
NW = 8            # windows per core
T = NW * W        # tokens per core
N_CORES = 8
SCALE = (E // 16) ** (-0.5)  # HEAD_DIM ** -0.5 = 0.125
NEG = -1.0e30


def build_nc(nw=NW):
    t_core = nw * W
    nc = bacc.Bacc("TRN2", target_bir_lowering=False, debug=False)

    x_d = nc.dram_tensor("x", [t_core, E], F32R, kind="ExternalInput")
    xt_d = nc.dram_tensor("xt", [E, t_core], F32R, kind="ExternalInput")
    w_d = {
        m: nc.dram_tensor(f"w{m}", [E, E], F32R, kind="ExternalInput")
        for m in ("q", "k", "v", "o")
    }
    bq_d = nc.dram_tensor("bq", [128, ET], F32, kind="ExternalInput")
    bk_d = nc.dram_tensor("bk", [128, ET], F32, kind="ExternalInput")
    bv_d = nc.dram_tensor("bv", [128, ET], F32, kind="ExternalInput")
    bo_d = nc.dram_tensor("bo", [1, E], F32R, kind="ExternalInput")
    o_d = nc.dram_tensor("o", [t_core, E], F32, kind="ExternalOutput")

    # host-side constants baked into the NEFF
    mask_np = np.zeros((2, 128, W), dtype=np.float32)
    for qt in range(2):
        r = np.arange(128)[:, None] + qt * 128
        c = np.arange(W)[None, :]
        mask_np[qt][c > r] = NEG
    mask_d = nc.inline_tensor(mask_np, "mask")
    ident_d = nc.inline_tensor(np.eye(128, dtype=np.float32), "ident")
    ones_d = nc.inline_tensor(np.ones((1, 128), dtype=np.float32), "ones")

    with tile.TileContext(nc) as tc:
        with (
            tc.tile_pool(name="wp", bufs=1) as wp,
            tc.tile_pool(name="cp", bufs=1) as cp,
            tc.tile_pool(name="xp", bufs=3) as xp,
            tc.tile_pool(name="xtp", bufs=2) as xtp,
            tc.tile_pool(name="qtp", bufs=1) as qtp,
            tc.tile_pool(name="ktp", bufs=1) as ktp,
            tc.tile_pool(name="otp", bufs=1) as otp,
            tc.tile_pool(name="vp", bufs=2) as vp,
            tc.tile_pool(name="sp", bufs=2) as sp,
            tc.tile_pool(name="ap_", bufs=2) as apool,
            tc.tile_pool(name="atp", bufs=2) as atp,
            tc.tile_pool(name="smp", bufs=4) as smp,
            tc.tile_pool(name="op", bufs=3) as op,
            tc.tile_pool(name="ps_qk", bufs=4, space=bass.MemorySpace.PSUM) as ps_qk,
            tc.tile_pool(name="ps_big", bufs=3, space=bass.MemorySpace.PSUM) as ps_big,
            tc.tile_pool(name="ps_tr", bufs=1, space=bass.MemorySpace.PSUM) as ps_tr,
        ):
            # ---- resident constants ----
            ident = cp.tile([128, 128], F32R, tag="ident")
            nc.gpsimd.dma_start(ident[:], ident_d.ap().bitcast(F32R))
            masks = cp.tile([128, 2, W], F32, tag="mask")
            for qt in range(2):
                nc.gpsimd.dma_start(masks[:, qt, :], mask_d.ap()[qt])
            ones = cp.tile([1, 128], F32R, tag="ones")
            nc.gpsimd.dma_start(ones[:], ones_d.ap().bitcast(F32R))
            bo_sb = cp.tile([1, E], F32R, tag="bo")
            nc.gpsimd.dma_start(bo_sb[:], bo_d.ap())
            bq_sb = cp.tile([128, ET], F32, tag="bq")
            nc.gpsimd.dma_start(bq_sb[:], bq_d.ap())
            bk_sb = cp.tile([128, ET], F32, tag="bk")
            nc.gpsimd.dma_start(bk_sb[:], bk_d.ap())
            bv_sb = cp.tile([128, ET], F32, tag="bv")
            nc.gpsimd.dma_start(bv_sb[:], bv_d.ap())

            # ---- resident weights: wsb[m][p, ei, eo] = W_m.T[ei*128+p, eo] ----
            # Weight DMAs are interleaved into window 0's emission below so
            # the sync engine starts x/window work immediately instead of
            # serializing 16MB of weight loads ahead of all compute.
            wsb = {}
            for m in ("q", "k", "v", "o"):
                wsb[m] = wp.tile([128, ET, E], F32R, tag=f"w{m}", name=f"w{m}sb")

            def load_weight(m, half=None, warm=False):
                # one 3D DMA per (half, ei-quadrant): 1MB transfers keep the
                # sync queue's ~0.6us/instr issue rate off the critical path
                wr = w_d[m].ap().rearrange("(a p) n -> a p n", p=128)
                halves = (0, 1) if half is None else (half,)
                for eoh in halves:
                    for eq in range(0, ET, 4):
                        nc.sync.dma_start(
                            wsb[m][:, eq : eq + 4, eoh * 512 : (eoh + 1) * 512],
                            wr[eq : eq + 4, :, eoh * 512 : (eoh + 1) * 512].transpose(
                                [1, 0, 2]
                            ),
                        )
                        if warm:
                            # keep the PE activity monitor warm through the
                            # DMA-bound phase: a tiny transpose per arriving
                            # chunk, paced by the DMA itself
                            wps = ps_tr.tile([128, 128], F32R, tag="tr", name="warm")
                            nc.tensor.transpose(
                                wps[:],
                                wsb[m][:, eq, eoh * 512 : eoh * 512 + 128],
                                ident[:],
                            )

            pending_stores = []

            def flush_stores():
                for dst, src_t in pending_stores:
                    nc.sync.dma_start(dst, src_t[:])
                pending_stores.clear()

            for w in range(nw):
                tok0 = w * W

                # ---- xT[p, ei, t] (e-major) loaded directly (host-transposed) ----
                xT = xtp.tile([128, ET, W], F32R, tag="xT")
                if w == 0:
                    # interleave xT and wq chunk loads so the first q-proj
                    # group starts as soon as the first chunk pair lands
                    wrq = w_d["q"].ap().rearrange("(a p) n -> a p n", p=128)
                    for ei in range(ET):
                        nc.sync.dma_start(
                            xT[:, ei, :],
                            xt_d.ap()[ei * 128 : (ei + 1) * 128, tok0 : tok0 + W],
                        )
                        nc.sync.dma_start(
                            wsb["q"][:, ei, 0:512], wrq[ei][:, 0:512]
                        )
                else:
                    xtr = xt_d.ap().rearrange("(a p) t -> a p t", p=128)
                    nc.sync.dma_start(
                        xT[:, :, :],
                        xtr[:, :, tok0 : tok0 + W].transpose([1, 0, 2]),
                    )
                # previous window's output stores go out behind our xT loads so
                # they never head-of-line-block the prefetch on the queue
                flush_stores()

                # ---- load x window (residual; not needed until o-proj) ----
                x_w = []
                if w > 0:
                    for tt in range(2):
                        xt_ = xp.tile([128, E], F32R, tag="x")
                        nc.sync.dma_start(
                            xt_[:], x_d.ap()[tok0 + tt * 128 : tok0 + (tt + 1) * 128, :]
                        )
                        x_w.append(xt_)

                if w == 0:
                    load_weight("q", half=1, warm=True)

                # ---- q/k projections -> [e_out, t] layout, bias fused ----
                qT = qtp.tile([128, ET, W], F32R, tag="qT")
                kT = ktp.tile([128, ET, W], F32R, tag="kT")
                for dst, m, b_sb in ((qT, "q", bq_sb), (kT, "k", bk_sb)):
                    if w == 0 and m == "k":
                        load_weight("k", half=1, warm=True)
                    for eo in range(ET):
                        if w == 0 and m == "q" and eo == 4:
                            load_weight("k", half=0, warm=True)
                        pp = ps_qk.tile([128, W], F32, tag="qk")
                        for ei in range(ET):
                            nc.tensor.matmul(
                                pp[:],
                                wsb[m][:, ei, eo * 128 : (eo + 1) * 128],
                                xT[:, ei, :],
                                start=(ei == 0),
                                stop=(ei == ET - 1),
                            )
                        nc.scalar.add(dst[:, eo, :], pp[:], b_sb[:, eo : eo + 1])

                if w == 0:
                    load_weight("v", half=0, warm=True)
                    for tt in range(2):
                        xt_ = xp.tile([128, E], F32R, tag="x", name="xt_w0")
                        nc.sync.dma_start(
                            xt_[:],
                            x_d.ap()[tok0 + tt * 128 : tok0 + (tt + 1) * 128, :],
                        )
                        x_w.append(xt_)

                # ---- scores + softmax + transpose(attn) ----
                aT = []
                for ktt in range(2):
                    t_ = atp.tile([128, W], F32R, tag="aT", name=f"aT{ktt}")
                    aT.append(t_)
                for qt in range(2):
                    sc = ps_qk.tile([128, W], F32, tag="qk")
                    for ei in range(ET):
                        nc.tensor.matmul(
                            sc[:],
                            qT[:, ei, qt * 128 : (qt + 1) * 128],
                            kT[:, ei, :],
                            start=(ei == 0),
                            stop=(ei == ET - 1),
                        )
                    s_sb = sp.tile([128, W], F32, tag="s")
                    nc.vector.tensor_add(s_sb[:], sc[:], masks[:, qt, :])
                    sums = smp.tile([128, 1], F32, tag="sum")
                    nc.scalar.activation(
                        s_sb[:], s_sb[:], AF.Exp, scale=SCALE, accum_out=sums[:]
                    )
                    rec = smp.tile([128, 1], F32, tag="rec")
                    nc.vector.reciprocal(rec[:], sums[:])
                    a_sb = apool.tile([128, W], F32R, tag="a")
                    nc.vector.tensor_scalar_mul(a_sb[:], s_sb[:], rec[:])
                    # transpose attn block rows->cols: aT[ktt][:, qt*128:...]
                    for ktt in range(2):
                        ptr = ps_tr.tile([128, 128], F32R, tag="tr", name="ptra")
                        nc.tensor.transpose(
                            ptr[:], a_sb[:, ktt * 128 : (ktt + 1) * 128], ident[:]
                        )
                        nc.vector.tensor_copy(
                            aT[ktt][:, qt * 128 : (qt + 1) * 128], ptr[:]
                        )

                if w == 0:
                    load_weight("o", half=0, warm=True)

                # ---- v projection (token-major) ----
                v_w = [vp.tile([128, E], F32R, tag="v", name=f"v{tt}") for tt in range(2)]
                for eoh in range(2):
                    if w == 0 and eoh == 1:
                        load_weight("v", half=1, warm=True)
                    for tt in range(2):
                        pv = ps_big.tile([128, 512], F32, tag="big")
                        for ei in range(ET):
                            nc.tensor.matmul(
                                pv[:],
                                xT[:, ei, tt * 128 : (tt + 1) * 128],
                                wsb["v"][:, ei, eoh * 512 : (eoh + 1) * 512],
                                start=(ei == 0),
                                stop=(ei == ET - 1),
                            )
                        nc.vector.tensor_copy(
                            v_w[tt][:, eoh * 512 : (eoh + 1) * 512], pv[:]
                        )

                # ---- attn @ v -> outT [e, t] layout, bias bv fused ----
                outT = otp.tile([128, ET, W], F32R, tag="outT")
                for et in range(ET):
                    pa = ps_qk.tile([128, W], F32, tag="qk")
                    for ktt in range(2):
                        nc.tensor.matmul(
                            pa[:],
                            v_w[ktt][:, et * 128 : (et + 1) * 128],
                            aT[ktt][:],
                            start=(ktt == 0),
                            stop=(ktt == 1),
                        )
                    nc.scalar.add(outT[:, et, :], pa[:], bv_sb[:, et : et + 1])

                # ---- output projection + bo + residual ----
                for eoh in range(2):
                    if w == 0 and eoh == 1:
                        load_weight("o", half=1, warm=True)
                    for tt in range(2):
                        po = ps_big.tile([128, 512], F32, tag="big")
                        for ei in range(ET):
                            nc.tensor.matmul(
                                po[:],
                                outT[:, ei, tt * 128 : (tt + 1) * 128],
                                wsb["o"][:, ei, eoh * 512 : (eoh + 1) * 512],
                                start=(ei == 0),
                                stop=False,
                            )
                        nc.tensor.matmul(
                            po[:],
                            ones[:],
                            bo_sb[:, eoh * 512 : (eoh + 1) * 512],
                            start=False,
                            stop=True,
                        )
                        o_sb = op.tile([128, 512], F32, tag="o")
                        nc.vector.tensor_add(
                            o_sb[:],
                            po[:],
                            x_w[tt][:, eoh * 512 : (eoh + 1) * 512].bitcast(F32),
                        )
                        dst_ap = o_d.ap()[
                            tok0 + tt * 128 : tok0 + (tt + 1) * 128,
                            eoh * 512 : (eoh + 1) * 512,
                        ]
                        if w == nw - 1:
                            nc.sync.dma_start(dst_ap, o_sb[:])
                        else:
                            pending_stores.append((dst_ap, o_sb))

            flush_stores()

    nc.compile()
    return nc


_NC_CACHE = {}


def _get_nc(nw=NW):
    if nw not in _NC_CACHE:
        _NC_CACHE[nw] = build_nc(nw)
    return _NC_CACHE[nw]


def kernel(x, Wq, bq, Wk, bk, Wv, bv, Wo, bo):
    x = np.asarray(x, dtype=np.float32)
    B, S, _ = x.shape
    x_flat = np.ascontiguousarray(x.reshape(B * S, E))
    t_core = B * S // N_CORES
    assert t_core == T

    common = {
        "wq": np.ascontiguousarray(np.asarray(Wq, np.float32).T),
        "wk": np.ascontiguousarray(np.asarray(Wk, np.float32).T),
        "wv": np.ascontiguousarray(np.asarray(Wv, np.float32).T),
        "wo": np.ascontiguousarray(np.asarray(Wo, np.float32).T),
        "bq": np.ascontiguousarray(np.asarray(bq, np.float32).reshape(ET, 128).T),
        "bk": np.ascontiguousarray(np.asarray(bk, np.float32).reshape(ET, 128).T),
        "bv": np.ascontiguousarray(np.asarray(bv, np.float32).reshape(ET, 128).T),
        "bo": np.ascontiguousarray(np.asarray(bo, np.float32).reshape(1, E)),
    }
    in_maps = [
        {
            "x": np.ascontiguousarray(x_flat[i * t_core : (i + 1) * t_core]),
            "xt": np.ascontiguousarray(x_flat[i * t_core : (i + 1) * t_core].T),
            **common,
        }
        for i in range(N_CORES)
    ]

    nc = _get_nc()
    res = run_bass_kernel_spmd(nc, in_maps, core_ids=list(range(N_CORES)))
    out = np.concatenate([res.results[i]["o"] for i in range(N_CORES)], axis=0)
    return out.reshape(B, S, E).astype(np.float32)



# revision 2
# speedup vs baseline: 1.7774x; 1.7774x over previous
"""Trainium2 Bass kernel for windowed (local) causal self-attention.

Reference computation (per batch element, fp32):
    q = x @ Wq.T + bq ; k = x @ Wk.T + bk ; v = x @ Wv.T + bv
    per non-overlapping window of 256 tokens:
        attn = softmax(causal_mask(q k^T * HEAD_DIM**-0.5))
        out  = attn @ v
    o = out @ Wo.T + bo + x

Algebraic reduction (the reference has no head split, so scores contract
over the full E=1024):
    q_i.k_j = x_i^T (Wq^T Wk) x_j + x_i.(Wq^T bk) + (Wk^T bq).x_j + bq.bk
The second and fourth terms are constant along the softmax axis and
cancel; with M = Wq^T Wk and vvec = Wk^T bq:
    scores = (X M + 1 vvec^T) X^T      (one projection instead of two)
Since softmax rows sum to 1,
    attn @ (X Wv^T + 1 bv^T) @ Wo^T + 1 bo^T = attn @ (X P^T) + 1 b'^T
with P = Wo Wv and b' = Wo bv + bo (one projection instead of two).
b' is folded into the residual copy of x on the host.  Device work per
token is therefore 2 E^2 MACs of projection + windowed attention, ~55%
of the naive PE work.

Sharding: data-parallel over (batch, window): 64 window-blocks of 256
tokens -> 8 cores x 8 windows.  M/P replicated (8MB instead of 16MB).

Per-core kernel strategy (all matmuls float32r: 1 cycle/row at N>=256):
  - M, P resident in SBUF; x streamed per window as host-transposed
    xT [E, 256] plus a token-major residual copy (x + b').
  - per window: yT = M^T-proj of xT in [e,t] layout (vvec bias fused in
    the ACT psum evac); scores = yT^T-blocks @ xT accumulated over 8
    e-tiles; causal mask added during psum evac; scale+exp+row-sum fused
    in one ACT op (accum_out); attn normalized by 1/sum on DVE and
    PE-transposed to attnT; Z = X P^T token-major; out = attnT^T @ Z
    directly token-major, residual (x + b') fused into the DVE evac.
  - window-0 weight DMA (8MB) is interleaved with compute emission and
    paced with tiny PE warmup transposes to keep the HAM clock at
    2.4GHz; output stores are deferred behind the next window's loads.
"""
import sys

sys.path.insert(0, "/opt/trn_rl_repo")

import numpy as np

import concourse.bass as bass
import concourse.bacc as bacc
import concourse.mybir as mybir
import concourse.tile as tile
from concourse.bass_utils import run_bass_kernel_spmd

F32 = mybir.dt.float32
F32R = mybir.dt.float32r
AF = mybir.ActivationFunctionType

E = 1024          # embed dim
ET = E // 128     # e-tiles
W = 256           # window size
NW = 8            # windows per core
T = NW * W        # tokens per core
N_CORES = 8
SCALE = (E // 16) ** (-0.5)  # HEAD_DIM ** -0.5 = 0.125
NEG = -1.0e30


def build_nc(nw=NW):
    t_core = nw * W
    nc = bacc.Bacc("TRN2", target_bir_lowering=False, debug=False)

    # x: token-major residual copy with b' = Wo@bv + bo pre-added (host)
    x_d = nc.dram_tensor("x", [t_core, E], F32R, kind="ExternalInput")
    xt_d = nc.dram_tensor("xt", [E, t_core], F32R, kind="ExternalInput")
    m_d = nc.dram_tensor("wm", [E, E], F32R, kind="ExternalInput")   # Wq^T @ Wk
    p_d = nc.dram_tensor("wz", [E, E], F32R, kind="ExternalInput")   # (Wo @ Wv)^T
    vv_d = nc.dram_tensor("vv", [128, ET], F32, kind="ExternalInput")  # Wk^T @ bq
    o_d = nc.dram_tensor("o", [t_core, E], F32, kind="ExternalOutput")

    # host-side constants baked into the NEFF
    mask_np = np.zeros((2, 128, W), dtype=np.float32)
    for qt in range(2):
        r = np.arange(128)[:, None] + qt * 128
        c = np.arange(W)[None, :]
        mask_np[qt][c > r] = NEG
    mask_d = nc.inline_tensor(mask_np, "mask")
    ident_d = nc.inline_tensor(np.eye(128, dtype=np.float32), "ident")

    with tile.TileContext(nc) as tc:
        with (
            tc.tile_pool(name="wgt", bufs=1) as wp,
            tc.tile_pool(name="cp", bufs=1) as cp,
            tc.tile_pool(name="xp", bufs=3) as xp,
            tc.tile_pool(name="xtp", bufs=2) as xtp,
            tc.tile_pool(name="ytp", bufs=2) as ytp,
            tc.tile_pool(name="zp", bufs=2) as zp,
            tc.tile_pool(name="sp", bufs=2) as sp,
            tc.tile_pool(name="ap_", bufs=2) as apool,
            tc.tile_pool(name="atp", bufs=2) as atp,
            tc.tile_pool(name="smp", bufs=4) as smp,
            tc.tile_pool(name="op", bufs=3) as op,
            tc.tile_pool(name="ps_qk", bufs=4, space=bass.MemorySpace.PSUM) as ps_qk,
            tc.tile_pool(name="ps_big", bufs=3, space=bass.MemorySpace.PSUM) as ps_big,
            tc.tile_pool(name="ps_tr", bufs=1, space=bass.MemorySpace.PSUM) as ps_tr,
        ):
            # ---- resident constants ----
            ident = cp.tile([128, 128], F32R, tag="ident")
            nc.gpsimd.dma_start(ident[:], ident_d.ap().bitcast(F32R))
            masks = cp.tile([128, 2, W], F32, tag="mask")
            for qt in range(2):
                nc.gpsimd.dma_start(masks[:, qt, :], mask_d.ap()[qt])
            vv_sb = cp.tile([128, ET], F32, tag="vv")
            nc.gpsimd.dma_start(vv_sb[:], vv_d.ap())

            # ---- resident weights: wsb[m][p, ei, eo] = Wm[ei*128+p, eo] ----
            wsb = {
                "m": wp.tile([128, ET, E], F32R, tag="wm", name="wmsb"),
                "z": wp.tile([128, ET, E], F32R, tag="wz", name="wzsb"),
            }
            w_d = {"m": m_d, "z": p_d}

            def load_weight(m, half=None, warm=False):
                # one 3D DMA per (half, ei-quadrant): 1MB transfers keep the
                # sync queue's ~0.6us/instr issue rate off the critical path
                wr = w_d[m].ap().rearrange("(a p) n -> a p n", p=128)
                halves = (0, 1) if half is None else (half,)
                for eoh in halves:
                    for eq in range(0, ET, 4):
                        nc.sync.dma_start(
                            wsb[m][:, eq : eq + 4, eoh * 512 : (eoh + 1) * 512],
                            wr[eq : eq + 4, :, eoh * 512 : (eoh + 1) * 512].transpose(
                                [1, 0, 2]
                            ),
                        )
                        if warm:
                            # keep the PE activity monitor warm through the
                            # DMA-bound phase: a tiny transpose per arriving
                            # chunk, paced by the DMA itself
                            wps = ps_tr.tile([128, 128], F32R, tag="tr", name="warm")
                            nc.tensor.transpose(
                                wps[:],
                                wsb[m][:, eq, eoh * 512 : eoh * 512 + 128],
                                ident[:],
                            )

            pending_stores = []

            def flush_stores():
                for dst, src_t in pending_stores:
                    nc.sync.dma_start(dst, src_t[:])
                pending_stores.clear()

            for w in range(nw):
                tok0 = w * W

                # ---- xT[p, ei, t] (e-major) loaded directly (host-transposed) ----
                xT = xtp.tile([128, ET, W], F32R, tag="xT")
                if w == 0:
                    for ei in range(ET):
                        nc.sync.dma_start(
                            xT[:, ei, :],
                            xt_d.ap()[ei * 128 : (ei + 1) * 128, tok0 : tok0 + W],
                        )
                    load_weight("m", half=0, warm=True)
                else:
                    xtr = xt_d.ap().rearrange("(a p) t -> a p t", p=128)
                    nc.sync.dma_start(
                        xT[:, :, :],
                        xtr[:, :, tok0 : tok0 + W].transpose([1, 0, 2]),
                    )
                # previous window's output stores go out behind our xT loads so
                # they never head-of-line-block the prefetch on the queue
                flush_stores()

                # ---- load x window (residual; not needed until out evac) ----
                x_w = []
                if w > 0:
                    for tt in range(2):
                        xt_ = xp.tile([128, E], F32R, tag="x")
                        nc.sync.dma_start(
                            xt_[:], x_d.ap()[tok0 + tt * 128 : tok0 + (tt + 1) * 128, :]
                        )
                        x_w.append(xt_)

                if w == 0:
                    load_weight("m", half=1, warm=True)

                # ---- y projection -> [e_out, t] layout, vvec bias fused ----
                yT = ytp.tile([128, ET, W], F32R, tag="yT")
                for eo in range(ET):
                    if w == 0 and eo == 4:
                        load_weight("z", half=0, warm=True)
                    pp = ps_qk.tile([128, W], F32, tag="qk")
                    for ei in range(ET):
                        nc.tensor.matmul(
                            pp[:],
                            wsb["m"][:, ei, eo * 128 : (eo + 1) * 128],
                            xT[:, ei, :],
                            start=(ei == 0),
                            stop=(ei == ET - 1),
                        )
                    nc.scalar.add(yT[:, eo, :], pp[:], vv_sb[:, eo : eo + 1])

                if w == 0:
                    load_weight("z", half=1, warm=True)
                    for tt in range(2):
                        xt_ = xp.tile([128, E], F32R, tag="x", name="xt_w0")
                        nc.sync.dma_start(
                            xt_[:],
                            x_d.ap()[tok0 + tt * 128 : tok0 + (tt + 1) * 128, :],
                        )
                        x_w.append(xt_)

                # ---- scores + softmax + transpose(attn) ----
                aT = []
                for ktt in range(2):
                    t_ = atp.tile([128, W], F32R, tag="aT", name=f"aT{ktt}")
                    aT.append(t_)
                for qt in range(2):
                    sc = ps_qk.tile([128, W], F32, tag="qk")
                    for ei in range(ET):
                        nc.tensor.matmul(
                            sc[:],
                            yT[:, ei, qt * 128 : (qt + 1) * 128],
                            xT[:, ei, :],
                            start=(ei == 0),
                            stop=(ei == ET - 1),
                        )
                    s_sb = sp.tile([128, W], F32, tag="s")
                    nc.vector.tensor_add(s_sb[:], sc[:], masks[:, qt, :])
                    sums = smp.tile([128, 1], F32, tag="sum")
                    nc.scalar.activation(
                        s_sb[:], s_sb[:], AF.Exp, scale=SCALE, accum_out=sums[:]
                    )
                    rec = smp.tile([128, 1], F32, tag="rec")
                    nc.vector.reciprocal(rec[:], sums[:])
                    a_sb = apool.tile([128, W], F32R, tag="a")
                    nc.vector.tensor_scalar_mul(a_sb[:], s_sb[:], rec[:])
                    # transpose attn block rows->cols: aT[ktt][:, qt*128:...]
                    for ktt in range(2):
                        ptr = ps_tr.tile([128, 128], F32R, tag="tr", name="ptra")
                        nc.tensor.transpose(
                            ptr[:], a_sb[:, ktt * 128 : (ktt + 1) * 128], ident[:]
                        )
                        nc.vector.tensor_copy(
                            aT[ktt][:, qt * 128 : (qt + 1) * 128], ptr[:]
                        )

                # ---- Z projection (token-major): Z = X P^T ----
                z_w = [zp.tile([128, E], F32R, tag="z", name=f"z{tt}") for tt in range(2)]
                for eoh in range(2):
                    for tt in range(2):
                        pv = ps_big.tile([128, 512], F32, tag="big")
                        for ei in range(ET):
                            nc.tensor.matmul(
                                pv[:],
                                xT[:, ei, tt * 128 : (tt + 1) * 128],
                                wsb["z"][:, ei, eoh * 512 : (eoh + 1) * 512],
                                start=(ei == 0),
                                stop=(ei == ET - 1),
                            )
                        nc.vector.tensor_copy(
                            z_w[tt][:, eoh * 512 : (eoh + 1) * 512], pv[:]
                        )

                # ---- out = attn @ Z directly token-major + residual ----
                for qt in range(2):
                    for eoh in range(2):
                        po = ps_big.tile([128, 512], F32, tag="big")
                        for ktt in range(2):
                            nc.tensor.matmul(
                                po[:],
                                aT[ktt][:, qt * 128 : (qt + 1) * 128],
                                z_w[ktt][:, eoh * 512 : (eoh + 1) * 512],
                                start=(ktt == 0),
                                stop=(ktt == 1),
                            )
                        o_sb = op.tile([128, 512], F32, tag="o")
                        nc.vector.tensor_add(
                            o_sb[:],
                            po[:],
                            x_w[qt][:, eoh * 512 : (eoh + 1) * 512].bitcast(F32),
                        )
                        dst_ap = o_d.ap()[
                            tok0 + qt * 128 : tok0 + (qt + 1) * 128,
                            eoh * 512 : (eoh + 1) * 512,
                        ]
                        if w == nw - 1:
                            nc.sync.dma_start(dst_ap, o_sb[:])
                        else:
                            pending_stores.append((dst_ap, o_sb))

            flush_stores()

    nc.compile()
    return nc


_NC_CACHE = {}


def _get_nc(nw=NW):
    if nw not in _NC_CACHE:
        _NC_CACHE[nw] = build_nc(nw)
    return _NC_CACHE[nw]


def _prep(x, Wq, bq, Wk, bk, Wv, bv, Wo, bo):
    """Host-side weight folding + per-core input maps."""
    x = np.asarray(x, dtype=np.float32)
    B, S, _ = x.shape
    Wq = np.asarray(Wq, np.float32)
    Wk = np.asarray(Wk, np.float32)
    Wv = np.asarray(Wv, np.float32)
    Wo = np.asarray(Wo, np.float32)
    bq = np.asarray(bq, np.float32)
    bv = np.asarray(bv, np.float32)
    bo = np.asarray(bo, np.float32)

    M = Wq.T @ Wk                      # scores = (X M) X^T  (+ col bias)
    Pt = (Wo @ Wv).T                   # Z = X @ Pt
    vvec = Wk.T @ bq                   # col bias, fused into y-projection
    bprime = Wo @ bv + bo              # folded into the residual below

    x_flat = x.reshape(B * S, E)
    x_resid = x_flat + bprime[None, :]
    t_core = B * S // N_CORES
    assert t_core == T

    common = {
        "wm": np.ascontiguousarray(M),
        "wz": np.ascontiguousarray(Pt),
        "vv": np.ascontiguousarray(vvec.reshape(ET, 128).T),
    }
    in_maps = [
        {
            "x": np.ascontiguousarray(x_resid[i * t_core : (i + 1) * t_core]),
            "xt": np.ascontiguousarray(x_flat[i * t_core : (i + 1) * t_core].T),
            **common,
        }
        for i in range(N_CORES)
    ]
    return in_maps


def kernel(x, Wq, bq, Wk, bk, Wv, bv, Wo, bo):
    in_maps = _prep(x, Wq, bq, Wk, bk, Wv, bv, Wo, bo)
    B, S = np.asarray(x).shape[:2]
    nc = _get_nc()
    res = run_bass_kernel_spmd(nc, in_maps, core_ids=list(range(N_CORES)))
    out = np.concatenate([res.results[i]["o"] for i in range(N_CORES)], axis=0)
    return out.reshape(B, S, E).astype(np.float32)
